# revision 1
# baseline (speedup 1.0000x reference)
"""Trainium2 Bass kernel for nn_AutoDim_75153337745779 (moe_routing).

Math (see reference):
  out[b,f,e] = sum_d gs[f,d]/4 * (y_d[b,f,e] - mu_d[e]) * rsig_d[e]
  y_d = einsum('bfi,fie->bfe', emb[:,:,:d], w_d);  mu/var over (b,f) per e.

Strategy (8 cores, data-parallel over batch):
  Phase 1 (device): per-core Gram matrices C_f = emb_f^T emb_f and column
    sums s_f via TensorE, accumulated in PSUM over the batch shard.
  Host: reduce partial stats over cores (exact), compute mu/var/rsig,
    gumbel-softmax gate, and fold everything into a single combined weight
    Wc[f,i,e] and bias[f,e]:
        out = emb @ Wc - bias
  Phase 2 (device): fused block-diagonal matmul out = emb @ Wc - bias.
    emb tiles are PE-transposed on chip so the contraction dim (i) lands on
    partitions; 4 fields are packed per 128-row group; fp32 matmuls (exact)
    stream 128-wide fe windows, bias is subtracted during the PSUM->SBUF
    copy, and 2-row-tile batched DMAs on both HWDGE queues keep the DMA
    engines saturated.

  Notes from HW bring-up:
  - float32r matmuls round the stationary operand aggressively (~7%% error
    on Gram); bf16 round-to-nearest inputs with fp32 PSUM accumulate are
    fine for statistics (error averages out), so the Gram runs in bf16.
  - PSUM has_written is cleared at bank granularity by a matmul's
    start=True, so each multi-step accumulation region must own a full
    bank; phase 1 splits the 10 Gram groups 5 in-loop + 5 post-loop.
"""
import sys
for _p in ("/opt/trn_rl_repo",):
    if _p not in sys.path:
        sys.path.insert(0, _p)

import numpy as np

import concourse.bacc as bacc
import concourse.bass as bass
import concourse.mybir as mybir
import concourse.tile as tile
from concourse.bass_utils import run_bass_kernel_spmd

B, F, E = 16384, 39, 32
IN_DIMS = (4, 8, 16, 32)
NC = 8
BC = B // NC            # 2048 rows per core
NT = BC // 128          # 16 tiles of 128 rows
G = 10                  # 40 padded fields / 4 per group
COLS = F * E            # 1248
PCOLS = G * 128         # 1280
F32 = mybir.dt.float32
F32R = mybir.dt.float32r
BF16 = mybir.dt.bfloat16

_CACHE = {}

# tunables (sim-sweepable)
TUNE = dict(p1_ebufs=4, p2_ebufs=3, p2_tsp=4, p2_osp=4, p2_tslab=3, p2_osb=3,
            p2_copy_engine="scalar", p2_alt=False)


def _build_phase1():
    nc = bacc.Bacc(None, target_bir_lowering=False)
    emb = nc.dram_tensor("emb", [BC, PCOLS], F32, kind="ExternalInput")
    ones_in = nc.dram_tensor("ones_in", [128, 1], BF16, kind="ExternalInput")
    c_out = nc.dram_tensor("c_out", [128, PCOLS], F32, kind="ExternalOutput")
    s_out = nc.dram_tensor("s_out", [1, PCOLS], F32, kind="ExternalOutput")

    with tile.TileContext(nc) as tc:
        with (
            tc.tile_pool(name="embp", bufs=TUNE["p1_ebufs"]) as embp,
            tc.tile_pool(name="erp", bufs=NT // 2) as erp,
            tc.tile_pool(name="misc", bufs=1) as misc,
            tc.tile_pool(name="outp", bufs=1) as outp,
        ):
            ones = misc.tile([128, 1], BF16, name="ones")
            nc.sync.dma_start(ones[:], ones_in[:, :])
            c_sb = outp.tile([128, PCOLS], F32, name="c_sb")
            s_sb = outp.tile([1, PCOLS], F32, name="s_sb")
            accp = tc.alloc_tile_pool(name="acc", bufs=1, space="PSUM")
            # one accumulating region per PSUM bank (multi-region banks lose
            # accumulation state when a later region's start clears the bank)
            gram5 = [accp.tile([128, 128], F32, name=f"gram{g}") for g in range(5)]
            ssum = [accp.tile([1, 512], F32, name=f"ssum{j}") for j in range(3)]

            ers = []
            for tt in range(NT // 2):
                e = embp.tile([128, 2 * PCOLS], F32, name="e", tag="e")
                src = emb[256 * tt: 256 * tt + 256, :].rearrange(
                    "(n p) m -> p n m", p=128)
                eng = nc.sync if tt % 2 == 0 else nc.scalar
                eng.dma_start(e[:].rearrange("p (n m) -> p n m", n=2), src)
                er = erp.tile([128, 2 * PCOLS], BF16, name="er", tag="er")
                nc.scalar.copy(er[:], e[:])
                ers.append(er)
                for n in range(2):
                    base = PCOLS * n
                    first = tt == 0 and n == 0
                    last = tt == NT // 2 - 1 and n == 1
                    for g in range(5):
                        blk = er[:, base + 128 * g: base + 128 * g + 128]
                        nc.tensor.matmul(gram5[g][:], blk, blk,
                                         start=first, stop=last)
                    for j in range(3):
                        w = 512 if j < 2 else 256
                        nc.tensor.matmul(ssum[j][:, 0:w], ones[:],
                                         er[:, base + 512 * j: base + 512 * j + w],
                                         start=first, stop=last)

            for g in range(5):
                nc.vector.tensor_copy(c_sb[:, 128 * g: 128 * g + 128], gram5[g][:])
            for j in range(3):
                w = 512 if j < 2 else 256
                nc.vector.tensor_copy(s_sb[:, 512 * j: 512 * j + w], ssum[j][:, 0:w])
            accp.release()
            # remaining groups: accumulate from resident bf16 tiles after the loop
            with tc.tile_pool(name="acc2", bufs=5, space="PSUM") as accp2:
                for g in range(5, G):
                    acc = accp2.tile([128, 128], F32, name="acc", tag="acc")
                    k = 0
                    for er in ers:
                        for n in range(2):
                            base = PCOLS * n
                            blk = er[:, base + 128 * g: base + 128 * g + 128]
                            nc.tensor.matmul(acc[:], blk, blk,
                                             start=(k == 0), stop=(k == NT - 1))
                            k += 1
                    nc.vector.tensor_copy(c_sb[:, 128 * g: 128 * g + 128], acc[:])
            nc.sync.dma_start(c_out[:, :], c_sb[:])
            nc.sync.dma_start(s_out[:, :], s_sb[:])
    nc.finalize()
    return nc


def _build_phase2():
    nc = bacc.Bacc(None, target_bir_lowering=False)
    emb = nc.dram_tensor("emb", [BC, PCOLS], F32, kind="ExternalInput")
    wbd = nc.dram_tensor("wbd", [128, G * 128], F32, kind="ExternalInput")
    bias = nc.dram_tensor("bias", [128, PCOLS], F32, kind="ExternalInput")
    ident = nc.dram_tensor("ident", [128, 128], F32, kind="ExternalInput")
    out = nc.dram_tensor("out", [BC, COLS], F32, kind="ExternalOutput")

    with tile.TileContext(nc) as tc:
        with (
            tc.tile_pool(name="embp", bufs=TUNE["p2_ebufs"]) as embp,
            tc.tile_pool(name="misc", bufs=1) as misc,
            tc.tile_pool(name="tsp", bufs=TUNE["p2_tsp"], space="PSUM") as tsp,
            tc.tile_pool(name="osp", bufs=TUNE["p2_osp"], space="PSUM") as osp,
            tc.tile_pool(name="tslab", bufs=TUNE["p2_tslab"]) as tslab,
            tc.tile_pool(name="osb", bufs=TUNE["p2_osb"]) as osbp,
        ):
            w_sb = misc.tile([128, G * 128], F32, name="w_sb")
            nc.sync.dma_start(w_sb[:], wbd[:, :])
            b_sb = misc.tile([128, PCOLS], F32, name="b_sb")
            nc.sync.dma_start(b_sb[:], bias[:, :])
            id_sb = misc.tile([128, 128], F32, name="id_sb")
            nc.sync.dma_start(id_sb[:], ident[:, :])

            for tt in range(NT // 2):
                e = embp.tile([128, 2 * PCOLS], F32, name="e", tag="e")
                src = emb[256 * tt: 256 * tt + 256, :].rearrange(
                    "(n p) m -> p n m", p=128)
                leng = nc.sync if (not TUNE["p2_alt"] or tt % 2 == 0) else nc.scalar
                leng.dma_start(e[:].rearrange("p (n m) -> p n m", n=2), src)
                o_sb = osbp.tile([128, 2 * PCOLS], F32, name="o_sb", tag="o_sb")

                for n in range(2):
                    base = PCOLS * n
                    # transpose groups of 4 fields: [128 b, 128 fi] -> [128 fi, 128 b]
                    slabs = []
                    for q in range(3):
                        ng = 4 if q < 2 else 2
                        tp = tsp.tile([128, 512], F32, name="tp", tag="tp")
                        for k in range(ng):
                            g = 4 * q + k
                            nc.tensor.transpose(tp[:, 128 * k: 128 * k + 128],
                                                e[:, base + 128 * g: base + 128 * g + 128],
                                                id_sb[:])
                        ts = tslab.tile([128, 512], F32, name="ts", tag="ts")
                        if TUNE["p2_copy_engine"] == "scalar":
                            nc.scalar.copy(ts[:, 0:128 * ng], tp[:, 0:128 * ng])
                        else:
                            nc.vector.tensor_copy(ts[:, 0:128 * ng], tp[:, 0:128 * ng])
                        slabs.append(ts)

                    o_ps = [osp.tile([128, 512], F32, name="ops", tag="ops")
                            for _ in range(3)]
                    for g in range(G):
                        dst = o_ps[g // 4][:, 128 * (g % 4): 128 * (g % 4) + 128]
                        lhsT = slabs[g // 4][:, 128 * (g % 4): 128 * (g % 4) + 128]
                        nc.tensor.matmul(dst, lhsT,
                                         w_sb[:, 128 * g: 128 * g + 128],
                                         start=True, stop=True)

                    for j in range(3):
                        w = 512 if j < 2 else 256
                        nc.vector.tensor_sub(o_sb[:, base + 512 * j: base + 512 * j + w],
                                             o_ps[j][:, 0:w],
                                             b_sb[:, 512 * j: 512 * j + w])
                dst = out[256 * tt: 256 * tt + 256, :].rearrange(
                    "(n p) m -> p n m", p=128)
                seng = nc.scalar if (not TUNE["p2_alt"] or tt % 2 == 0) else nc.sync
                seng.dma_start(
                    dst, o_sb[:].rearrange("p (n m) -> p n m", n=2)[:, :, 0:COLS])
    nc.finalize()
    return nc


def _host_fold(Cg, Sg, w4, w8, w16, w32, gate, noise_u):
    ws = {4: w4, 8: w8, 16: w16, 32: w32}
    C_f = np.zeros((F, 32, 32), np.float64)
    for f in range(F):
        g, a = f // 4, f % 4
        C_f[f] = Cg[32 * a:32 * a + 32, 128 * g + 32 * a:128 * g + 32 * a + 32]
    s_f = Sg.reshape(G * 4, 32)[:F].astype(np.float64)

    mu = np.zeros((4, E)); msq = np.zeros((4, E))
    for k, d in enumerate(IN_DIMS):
        w = ws[d].astype(np.float64)
        mu[k] = np.einsum('fi,fie->e', s_f[:, :d], w) / (B * F)
        msq[k] = np.einsum('fij,fie,fje->e', C_f[:, :d, :d], w, w) / (B * F)
    var = msq - mu ** 2
    rsig = 1.0 / np.sqrt(var + 1e-5)

    gmb = -np.log(-np.log(noise_u.astype(np.float64) + 1e-10) + 1e-10)
    z = (gate.astype(np.float64) + gmb)
    z -= z.max(axis=-1, keepdims=True)
    gs = np.exp(z) / np.exp(z).sum(axis=-1, keepdims=True)
    a_ = gs / 4.0

    Wc = np.zeros((F, 32, E), np.float64)
    bias = np.zeros((F, E), np.float64)
    for k, d in enumerate(IN_DIMS):
        w = ws[d].astype(np.float64)
        Wc[:, :d, :] += a_[:, k, None, None] * rsig[k][None, None, :] * w
        bias += a_[:, k, None] * (rsig[k] * mu[k])[None, :]

    Wbd = np.zeros((G, 128, 128), np.float32)
    bias_pc = np.zeros((128, PCOLS), np.float32)
    for f in range(F):
        g, a = f // 4, f % 4
        Wbd[g, 32 * a:32 * a + 32, 32 * a:32 * a + 32] = Wc[f]
        bias_pc[:, 128 * g + 32 * a: 128 * g + 32 * a + 32] = bias[f][None, :]
    return Wbd, bias_pc


def kernel(emb, w4, w8, w16, w32, gate, noise_u):
    emb = np.asarray(emb, np.float32).reshape(B, COLS)
    embp = np.zeros((B, PCOLS), np.float32)
    embp[:, :COLS] = emb
    shards = embp.reshape(NC, BC, PCOLS)
    core_ids = list(range(NC))

    if "p1" not in _CACHE:
        _CACHE["p1"] = _build_phase1()
    import ml_dtypes
    ones_in = np.ones((128, 1), ml_dtypes.bfloat16)
    r1 = run_bass_kernel_spmd(
        _CACHE["p1"],
        [{"emb": shards[c], "ones_in": ones_in} for c in range(NC)],
        core_ids,
    ).results
    Cg = np.zeros((128, PCOLS), np.float64)
    Sg = np.zeros((1, PCOLS), np.float64)
    for r in r1:
        Cg += r["c_out"]
        Sg += r["s_out"]

    Wbd, bias_pc = _host_fold(Cg, Sg, np.asarray(w4), np.asarray(w8),
                              np.asarray(w16), np.asarray(w32),
                              np.asarray(gate), np.asarray(noise_u))
    Wbd = np.ascontiguousarray(Wbd.transpose(1, 0, 2).reshape(128, G * 128))
    ident = np.eye(128, dtype=np.float32)

    if "p2" not in _CACHE:
        _CACHE["p2"] = _build_phase2()
    r2 = run_bass_kernel_spmd(
        _CACHE["p2"],
        [{"emb": shards[c], "wbd": Wbd, "bias": bias_pc, "ident": ident}
         for c in range(NC)],
        core_ids,
    ).results
    out = np.concatenate([r["out"] for r in r2], axis=0)
    return out.reshape(B, F, E)



# revision 2
# speedup vs baseline: 1.3216x; 1.3216x over previous
"""Trainium2 Bass kernel for nn_AutoDim_75153337745779 (moe_routing).

Math (see reference):
  out[b,f,e] = sum_d gs[f,d]/4 * (y_d[b,f,e] - mu_d[e]) * rsig_d[e]
  y_d = einsum('bfi,fie->bfe', emb[:,:,:d], w_d);  mu/var over (b,f) per e.

Strategy (8 cores, data-parallel over batch):
  Phase 1 (device): per-core Gram matrices C_f = emb_f^T emb_f and column
    sums s_f via TensorE, accumulated in PSUM over the batch shard.
  Host: reduce partial stats over cores (exact), compute mu/var/rsig,
    gumbel-softmax gate, and fold everything into a single combined weight
    Wc[f,i,e] and bias[f,e]:
        out = emb @ Wc - bias
  Phase 2 (device): fused block-diagonal matmul out = emb @ Wc - bias.
    emb tiles are PE-transposed on chip so the contraction dim (i) lands on
    partitions; 4 fields are packed per 128-row group; bias is subtracted
    during the PSUM->SBUF copy, and 2-row-tile batched DMAs keep the DMA
    engines saturated.

  Precision plan (tolerance is 2e-2 max-rel; measured headroom is ~4000x):
  - emb is converted to bf16 on the host, halving all input DMA traffic and
    letting every PE op run at 1 cycle/row (fp32 is 4, fp32 transpose 2).
  - Gram/colsum statistics in bf16 accumulate into fp32 PSUM; rounding is
    zero-mean and averages out over the 16k-row batch shard.
  - Phase-2 matmul emb_bf16 @ Wc_bf16 with fp32 PSUM; bias stays fp32.
"""
import sys
for _p in ("/opt/trn_rl_repo",):
    if _p not in sys.path:
        sys.path.insert(0, _p)

import numpy as np

import concourse.bacc as bacc
import concourse.bass as bass
import concourse.mybir as mybir
import concourse.tile as tile
from concourse.bass_utils import run_bass_kernel_spmd

B, F, E = 16384, 39, 32
IN_DIMS = (4, 8, 16, 32)
NC = 8
BC = B // NC            # 2048 rows per core
NT = BC // 128          # 16 tiles of 128 rows
G = 10                  # 40 padded fields / 4 per group
COLS = F * E            # 1248
PCOLS = G * 128         # 1280
F32 = mybir.dt.float32
BF16 = mybir.dt.bfloat16

_CACHE = {}

# tunables (sim-sweepable)
TUNE = dict(p1_ebufs=8, p2_ebufs=3, p2_tsp=4, p2_osp=4, p2_tslab=3, p2_osb=3,
            p2_copy_engine="scalar", p2_alt=False)


def _build_phase1():
    nc = bacc.Bacc(None, target_bir_lowering=False)
    emb = nc.dram_tensor("emb", [BC, PCOLS], BF16, kind="ExternalInput")
    ones_in = nc.dram_tensor("ones_in", [128, 1], BF16, kind="ExternalInput")
    c_out = nc.dram_tensor("c_out", [128, PCOLS], F32, kind="ExternalOutput")
    s_out = nc.dram_tensor("s_out", [1, PCOLS], F32, kind="ExternalOutput")

    with tile.TileContext(nc) as tc:
        with (
            tc.tile_pool(name="embp", bufs=TUNE["p1_ebufs"]) as embp,
            tc.tile_pool(name="misc", bufs=1) as misc,
            tc.tile_pool(name="outp", bufs=1) as outp,
        ):
            ones = misc.tile([128, 1], BF16, name="ones")
            nc.sync.dma_start(ones[:], ones_in[:, :])
            c_sb = outp.tile([128, PCOLS], F32, name="c_sb")
            s_sb = outp.tile([1, PCOLS], F32, name="s_sb")
            accp = tc.alloc_tile_pool(name="acc", bufs=1, space="PSUM")
            # one accumulating region per PSUM bank (multi-region banks lose
            # accumulation state when a later region's start clears the bank)
            gram5 = [accp.tile([128, 128], F32, name=f"gram{g}") for g in range(5)]
            ssum = [accp.tile([1, 512], F32, name=f"ssum{j}") for j in range(3)]

            es = []
            for tt in range(NT // 2):
                e = embp.tile([128, 2 * PCOLS], BF16, name="e", tag="e")
                src = emb[256 * tt: 256 * tt + 256, :].rearrange(
                    "(n p) m -> p n m", p=128)
                eng = nc.sync if tt % 2 == 0 else nc.scalar
                eng.dma_start(e[:].rearrange("p (n m) -> p n m", n=2), src)
                es.append(e)
                for n in range(2):
                    base = PCOLS * n
                    first = tt == 0 and n == 0
                    last = tt == NT // 2 - 1 and n == 1
                    for g in range(5):
                        blk = e[:, base + 128 * g: base + 128 * g + 128]
                        nc.tensor.matmul(gram5[g][:], blk, blk,
                                         start=first, stop=last)
                    for j in range(3):
                        w = 512 if j < 2 else 256
                        nc.tensor.matmul(ssum[j][:, 0:w], ones[:],
                                         e[:, base + 512 * j: base + 512 * j + w],
                                         start=first, stop=last)

            for g in range(5):
                nc.vector.tensor_copy(c_sb[:, 128 * g: 128 * g + 128], gram5[g][:])
            for j in range(3):
                w = 512 if j < 2 else 256
                nc.vector.tensor_copy(s_sb[:, 512 * j: 512 * j + w], ssum[j][:, 0:w])
            accp.release()
            # remaining groups: accumulate from resident bf16 tiles after the loop
            with tc.tile_pool(name="acc2", bufs=5, space="PSUM") as accp2:
                for g in range(5, G):
                    acc = accp2.tile([128, 128], F32, name="acc", tag="acc")
                    k = 0
                    for e in es:
                        for n in range(2):
                            base = PCOLS * n
                            blk = e[:, base + 128 * g: base + 128 * g + 128]
                            nc.tensor.matmul(acc[:], blk, blk,
                                             start=(k == 0), stop=(k == NT - 1))
                            k += 1
                    nc.vector.tensor_copy(c_sb[:, 128 * g: 128 * g + 128], acc[:])
            nc.sync.dma_start(c_out[:, :], c_sb[:])
            nc.sync.dma_start(s_out[:, :], s_sb[:])
    nc.finalize()
    return nc


def _build_phase2():
    nc = bacc.Bacc(None, target_bir_lowering=False)
    emb = nc.dram_tensor("emb", [BC, PCOLS], BF16, kind="ExternalInput")
    wbd = nc.dram_tensor("wbd", [128, G * 128], BF16, kind="ExternalInput")
    bias = nc.dram_tensor("bias", [128, PCOLS], F32, kind="ExternalInput")
    ident = nc.dram_tensor("ident", [128, 128], BF16, kind="ExternalInput")
    out = nc.dram_tensor("out", [BC, COLS], F32, kind="ExternalOutput")

    with tile.TileContext(nc) as tc:
        with (
            tc.tile_pool(name="embp", bufs=TUNE["p2_ebufs"]) as embp,
            tc.tile_pool(name="misc", bufs=1) as misc,
            tc.tile_pool(name="tsp", bufs=TUNE["p2_tsp"], space="PSUM") as tsp,
            tc.tile_pool(name="osp", bufs=TUNE["p2_osp"], space="PSUM") as osp,
            tc.tile_pool(name="tslab", bufs=TUNE["p2_tslab"]) as tslab,
            tc.tile_pool(name="osb", bufs=TUNE["p2_osb"]) as osbp,
        ):
            w_sb = misc.tile([128, G * 128], BF16, name="w_sb")
            nc.scalar.dma_start(w_sb[:], wbd[:, :])
            b_sb = misc.tile([128, PCOLS], F32, name="b_sb")
            nc.scalar.dma_start(b_sb[:], bias[:, :])
            id_sb = misc.tile([128, 128], BF16, name="id_sb")
            nc.scalar.dma_start(id_sb[:], ident[:, :])

            for tt in range(NT // 2):
                e = embp.tile([128, 2 * PCOLS], BF16, name="e", tag="e")
                src = emb[256 * tt: 256 * tt + 256, :].rearrange(
                    "(n p) m -> p n m", p=128)
                leng = nc.sync if (not TUNE["p2_alt"] or tt % 2 == 0) else nc.scalar
                leng.dma_start(e[:].rearrange("p (n m) -> p n m", n=2), src)
                o_sb = osbp.tile([128, 2 * PCOLS], F32, name="o_sb", tag="o_sb")

                for n in range(2):
                    base = PCOLS * n
                    # transpose groups of 4 fields: [128 b, 128 fi] -> [128 fi, 128 b]
                    slabs = []
                    for q in range(3):
                        ng = 4 if q < 2 else 2
                        tp = tsp.tile([128, 512], BF16, name="tp", tag="tp")
                        for k in range(ng):
                            g = 4 * q + k
                            nc.tensor.transpose(tp[:, 128 * k: 128 * k + 128],
                                                e[:, base + 128 * g: base + 128 * g + 128],
                                                id_sb[:])
                        ts = tslab.tile([128, 512], BF16, name="ts", tag="ts")
                        if TUNE["p2_copy_engine"] == "scalar":
                            nc.scalar.copy(ts[:, 0:128 * ng], tp[:, 0:128 * ng])
                        else:
                            nc.vector.tensor_copy(ts[:, 0:128 * ng], tp[:, 0:128 * ng])
                        slabs.append(ts)

                    o_ps = [osp.tile([128, 512], F32, name="ops", tag="ops")
                            for _ in range(3)]
                    for g in range(G):
                        dst = o_ps[g // 4][:, 128 * (g % 4): 128 * (g % 4) + 128]
                        lhsT = slabs[g // 4][:, 128 * (g % 4): 128 * (g % 4) + 128]
                        nc.tensor.matmul(dst, lhsT,
                                         w_sb[:, 128 * g: 128 * g + 128],
                                         start=True, stop=True)

                    for j in range(3):
                        w = 512 if j < 2 else 256
                        nc.vector.tensor_sub(o_sb[:, base + 512 * j: base + 512 * j + w],
                                             o_ps[j][:, 0:w],
                                             b_sb[:, 512 * j: 512 * j + w])
                dst = out[256 * tt: 256 * tt + 256, :].rearrange(
                    "(n p) m -> p n m", p=128)
                seng = nc.scalar if (not TUNE["p2_alt"] or tt % 2 == 0) else nc.sync
                seng.dma_start(
                    dst, o_sb[:].rearrange("p (n m) -> p n m", n=2)[:, :, 0:COLS])
    nc.finalize()
    return nc


def _host_fold(Cg, Sg, w4, w8, w16, w32, gate, noise_u):
    ws = {4: w4, 8: w8, 16: w16, 32: w32}
    C_f = np.zeros((F, 32, 32), np.float64)
    for f in range(F):
        g, a = f // 4, f % 4
        C_f[f] = Cg[32 * a:32 * a + 32, 128 * g + 32 * a:128 * g + 32 * a + 32]
    s_f = Sg.reshape(G * 4, 32)[:F].astype(np.float64)

    mu = np.zeros((4, E)); msq = np.zeros((4, E))
    for k, d in enumerate(IN_DIMS):
        w = ws[d].astype(np.float64)
        mu[k] = np.einsum('fi,fie->e', s_f[:, :d], w) / (B * F)
        msq[k] = np.einsum('fij,fie,fje->e', C_f[:, :d, :d], w, w) / (B * F)
    var = msq - mu ** 2
    rsig = 1.0 / np.sqrt(var + 1e-5)

    gmb = -np.log(-np.log(noise_u.astype(np.float64) + 1e-10) + 1e-10)
    z = (gate.astype(np.float64) + gmb)
    z -= z.max(axis=-1, keepdims=True)
    gs = np.exp(z) / np.exp(z).sum(axis=-1, keepdims=True)
    a_ = gs / 4.0

    Wc = np.zeros((F, 32, E), np.float64)
    bias = np.zeros((F, E), np.float64)
    for k, d in enumerate(IN_DIMS):
        w = ws[d].astype(np.float64)
        Wc[:, :d, :] += a_[:, k, None, None] * rsig[k][None, None, :] * w
        bias += a_[:, k, None] * (rsig[k] * mu[k])[None, :]

    Wbd = np.zeros((G, 128, 128), np.float32)
    bias_pc = np.zeros((128, PCOLS), np.float32)
    for f in range(F):
        g, a = f // 4, f % 4
        Wbd[g, 32 * a:32 * a + 32, 32 * a:32 * a + 32] = Wc[f]
        bias_pc[:, 128 * g + 32 * a: 128 * g + 32 * a + 32] = bias[f][None, :]
    return Wbd, bias_pc


def kernel(emb, w4, w8, w16, w32, gate, noise_u):
    import ml_dtypes
    emb = np.asarray(emb, np.float32).reshape(B, COLS)
    embp = np.zeros((B, PCOLS), ml_dtypes.bfloat16)
    embp[:, :COLS] = emb
    shards = embp.reshape(NC, BC, PCOLS)
    core_ids = list(range(NC))

    if "p1" not in _CACHE:
        _CACHE["p1"] = _build_phase1()
    ones_in = np.ones((128, 1), ml_dtypes.bfloat16)
    r1 = run_bass_kernel_spmd(
        _CACHE["p1"],
        [{"emb": shards[c], "ones_in": ones_in} for c in range(NC)],
        core_ids,
    ).results
    Cg = np.zeros((128, PCOLS), np.float64)
    Sg = np.zeros((1, PCOLS), np.float64)
    for r in r1:
        Cg += r["c_out"]
        Sg += r["s_out"]

    Wbd, bias_pc = _host_fold(Cg, Sg, np.asarray(w4), np.asarray(w8),
                              np.asarray(w16), np.asarray(w32),
                              np.asarray(gate), np.asarray(noise_u))
    Wbd = np.ascontiguousarray(
        Wbd.transpose(1, 0, 2).reshape(128, G * 128)).astype(ml_dtypes.bfloat16)
    ident = np.eye(128, dtype=ml_dtypes.bfloat16)

    if "p2" not in _CACHE:
        _CACHE["p2"] = _build_phase2()
    r2 = run_bass_kernel_spmd(
        _CACHE["p2"],
        [{"emb": shards[c], "wbd": Wbd, "bias": bias_pc, "ident": ident}
         for c in range(NC)],
        core_ids,
    ).results
    out = np.concatenate([r["out"] for r in r2], axis=0)
    return out.reshape(B, F, E)


# revision 22
# speedup vs baseline: 1.8349x; 1.3884x over previous
"""Trainium2 Bass kernel for nn_AutoDim_75153337745779 (moe_routing).

Math (see reference):
  out[b,f,e] = sum_d gs[f,d]/4 * (y_d[b,f,e] - mu_d[e]) * rsig_d[e]
  y_d = einsum('bfi,fie->bfe', emb[:,:,:d], w_d);  mu/var over (b,f) per e.

Strategy (8 cores, data-parallel over batch):
  Phase 1 (device): per-core Gram matrices C_f = emb_f^T emb_f and column
    sums s_f via TensorE, accumulated in PSUM over the batch shard.
  Host: reduce partial stats over cores (exact), compute mu/var/rsig,
    gumbel-softmax gate, and fold everything into a single combined weight
    Wc[f,i,e] and bias[f,e]:
        out = emb @ Wc - bias
  Phase 2 (device): fused block-diagonal matmul out = emb @ Wc - bias.
    emb tiles are PE-transposed on chip so the contraction dim (i) lands on
    partitions; 4 fields are packed per 128-row group; bias is subtracted
    during the PSUM->SBUF copy, and 2-row-tile batched DMAs keep the DMA
    engines saturated.

  Precision plan (tolerance is 2e-2 max-rel; measured headroom is ~4000x):
  - Phase 1 reads emb as fp8e4m3 and runs Gram/colsum in DoubleRow perf
    mode (2 k-tiles = 256 rows per matmul at 0.5 cyc/row). fp8 rounding is
    zero-mean and averages out over the 16k-row batch, so the statistics
    stay accurate to ~1e-3 relative.
  - Phase 2 reads emb as bf16 (host-converted): all PE ops at 1 cycle/row
    (fp32 is 4, fp32 transpose 2) and input DMA halves. Matmul
    emb_bf16 @ Wc_bf16 accumulates in fp32 PSUM; bias stays fp32.
"""
import sys
for _p in ("/opt/trn_rl_repo",):
    if _p not in sys.path:
        sys.path.insert(0, _p)

import numpy as np

import concourse.bacc as bacc
import concourse.bass as bass
import concourse.mybir as mybir
import concourse.tile as tile
from concourse.bass_utils import run_bass_kernel_spmd

B, F, E = 16384, 39, 32
IN_DIMS = (4, 8, 16, 32)
NC = 8
BC = B // NC            # 2048 rows per core
NT = BC // 128          # 16 tiles of 128 rows
G = 10                  # 40 padded fields / 4 per group
COLS = F * E            # 1248
PCOLS = G * 128         # 1280
F32 = mybir.dt.float32
BF16 = mybir.dt.bfloat16
FP8 = mybir.dt.float8e4
DR = mybir.MatmulPerfMode.DoubleRow

_CACHE = {}

# tunables (sim-sweepable)
TUNE = dict(p1_ebufs=8, p2_ebufs=8, p2_tsp=4, p2_osp=4, p2_tslab=3, p2_osb=6,
            p2_copy_engine="scalar")


def _build_phase1():
    nc = bacc.Bacc(None, target_bir_lowering=False)
    emb = nc.dram_tensor("emb", [BC, PCOLS], FP8, kind="ExternalInput")
    # 32 identical ones columns: DoubleRow ldweights rejects a 1-wide
    # stationary, and a 32-wide one costs the same (cost scales with the
    # moving free dim only). Row 0 of the [32, w] colsum result is used.
    ones_in = nc.dram_tensor("ones_in", [128, 64], FP8, kind="ExternalInput")
    c_out = nc.dram_tensor("c_out", [128, PCOLS], F32, kind="ExternalOutput")
    s_out = nc.dram_tensor("s_out", [1, PCOLS], F32, kind="ExternalOutput")

    with tile.TileContext(nc) as tc:
        with (
            tc.tile_pool(name="embp", bufs=TUNE["p1_ebufs"]) as embp,
            tc.tile_pool(name="misc", bufs=1) as misc,
            tc.tile_pool(name="outp", bufs=1) as outp,
        ):
            ones = misc.tile([128, 64], FP8, name="ones")
            nc.gpsimd.dma_start(ones[:], ones_in[:, :])
            onev = ones[:].rearrange("p (n m) -> p n m", n=2)
            c_sb = outp.tile([128, PCOLS], F32, name="c_sb")
            s_sb = outp.tile([1, PCOLS], F32, name="s_sb")
            # one accumulating region per PSUM bank (multi-region banks lose
            # accumulation state when a later region's start clears the bank).
            # gram and ssum live in separate pools so the gram banks release
            # (and the post-loop groups start) without waiting on ssum copies.
            accs = tc.alloc_tile_pool(name="accs", bufs=1, space="PSUM")
            accg = tc.alloc_tile_pool(name="accg", bufs=1, space="PSUM")
            ssum = [accs.tile([32, 512], F32, name=f"ssum{j}") for j in range(3)]
            gram5 = [accg.tile([128, 128], F32, name=f"gram{g}") for g in range(5)]

            es = []
            for tt in range(NT // 2):
                e = embp.tile([128, 2 * PCOLS], FP8, name="e", tag="e")
                src = emb[256 * tt: 256 * tt + 256, :].rearrange(
                    "(n p) m -> p n m", p=128)
                eng = nc.sync if tt % 2 == 0 else nc.scalar
                ev = e[:].rearrange("p (n m) -> p n m", n=2)
                eng.dma_start(ev, src)
                es.append(ev)
                first = tt == 0
                last = tt == NT // 2 - 1
                # DoubleRow: both 128-row k-tiles of this load in one matmul
                for g in range(5):
                    blk = ev[:, :, 128 * g: 128 * g + 128]
                    nc.tensor.matmul(gram5[g][:], blk, blk,
                                     start=first, stop=last, perf_mode=DR)
                for j in range(3):
                    w = 512 if j < 2 else 256
                    nc.tensor.matmul(ssum[j][:, 0:w], onev,
                                     ev[:, :, 512 * j: 512 * j + w],
                                     start=first, stop=last, perf_mode=DR)

            for g in range(5):
                ceng = nc.vector.tensor_copy if g % 2 == 0 else nc.scalar.copy
                ceng(c_sb[:, 128 * g: 128 * g + 128], gram5[g][:])
            accg.release()
            # first half of C ships while the post-loop groups accumulate
            nc.sync.dma_start(c_out[:, 0:640], c_sb[:, 0:640])
            for j in range(3):
                w = 512 if j < 2 else 256
                nc.scalar.copy(s_sb[:, 512 * j: 512 * j + w], ssum[j][0:1, 0:w])
            accs.release()
            nc.scalar.dma_start(s_out[:, :], s_sb[:])
            # remaining groups: accumulate from resident fp8 tiles after the loop
            with tc.tile_pool(name="acc2", bufs=5, space="PSUM") as accp2:
                for g in range(5, G):
                    acc = accp2.tile([128, 128], F32, name="acc", tag="acc")
                    for k, ev in enumerate(es):
                        blk = ev[:, :, 128 * g: 128 * g + 128]
                        nc.tensor.matmul(acc[:], blk, blk,
                                         start=(k == 0), stop=(k == len(es) - 1),
                                         perf_mode=DR)
                    ceng = nc.vector.tensor_copy if g % 2 == 0 else nc.scalar.copy
                    ceng(c_sb[:, 128 * g: 128 * g + 128], acc[:])
                    if g == 8:
                        nc.sync.dma_start(c_out[:, 640:1152], c_sb[:, 640:1152])
                    elif g == 9:
                        nc.scalar.dma_start(c_out[:, 1152:PCOLS], c_sb[:, 1152:PCOLS])
    nc.finalize()
    return nc


def _build_phase2():
    nc = bacc.Bacc(None, target_bir_lowering=False)
    emb = nc.dram_tensor("emb", [BC, PCOLS], BF16, kind="ExternalInput")
    wbd = nc.dram_tensor("wbd", [128, G * 128], BF16, kind="ExternalInput")
    bias = nc.dram_tensor("bias", [128, PCOLS], F32, kind="ExternalInput")
    ident = nc.dram_tensor("ident", [128, 128], BF16, kind="ExternalInput")
    out = nc.dram_tensor("out", [BC, COLS], F32, kind="ExternalOutput")

    with tile.TileContext(nc) as tc:
        with (
            tc.tile_pool(name="embp", bufs=TUNE["p2_ebufs"]) as embp,
            tc.tile_pool(name="misc", bufs=1) as misc,
            tc.tile_pool(name="tsp", bufs=TUNE["p2_tsp"], space="PSUM") as tsp,
            tc.tile_pool(name="osp", bufs=TUNE["p2_osp"], space="PSUM") as osp,
            tc.tile_pool(name="tslab", bufs=TUNE["p2_tslab"]) as tslab,
            tc.tile_pool(name="osb", bufs=TUNE["p2_osb"]) as osbp,
        ):
            id_sb = misc.tile([128, 128], BF16, name="id_sb")
            nc.scalar.dma_start(id_sb[:], ident[:, :])
            w_sb = misc.tile([128, G * 128], BF16, name="w_sb")
            nc.scalar.dma_start(w_sb[:], wbd[:, :])
            b_sb = misc.tile([128, PCOLS], F32, name="b_sb")
            nc.scalar.dma_start(b_sb[:], bias[:, :])

            for tt in range(NT // 2):
                e = embp.tile([128, 2 * PCOLS], BF16, name="e", tag="e")
                src = emb[256 * tt: 256 * tt + 256, :].rearrange(
                    "(n p) m -> p n m", p=128)
                nc.sync.dma_start(e[:].rearrange("p (n m) -> p n m", n=2), src)

                for n in range(2):
                    base = PCOLS * n
                    # transpose groups of 4 fields: [128 b, 128 fi] -> [128 fi, 128 b]
                    slabs = []
                    for q in range(3):
                        ng = 4 if q < 2 else 2
                        tp = tsp.tile([128, 512], BF16, name="tp", tag="tp")
                        for k in range(ng):
                            g = 4 * q + k
                            nc.tensor.transpose(tp[:, 128 * k: 128 * k + 128],
                                                e[:, base + 128 * g: base + 128 * g + 128],
                                                id_sb[:])
                        ts = tslab.tile([128, 512], BF16, name="ts", tag="ts")
                        if TUNE["p2_copy_engine"] == "scalar":
                            nc.scalar.copy(ts[:, 0:128 * ng], tp[:, 0:128 * ng])
                        else:
                            nc.vector.tensor_copy(ts[:, 0:128 * ng], tp[:, 0:128 * ng])
                        slabs.append(ts)

                    o_ps = [osp.tile([128, 512], F32, name="ops", tag="ops")
                            for _ in range(3)]
                    for g in range(G):
                        dst = o_ps[g // 4][:, 128 * (g % 4): 128 * (g % 4) + 128]
                        lhsT = slabs[g // 4][:, 128 * (g % 4): 128 * (g % 4) + 128]
                        nc.tensor.matmul(dst, lhsT,
                                         w_sb[:, 128 * g: 128 * g + 128],
                                         start=True, stop=True)

                    o_sb = osbp.tile([128, PCOLS], F32, name="o_sb", tag="o_sb")
                    for j in range(3):
                        w = 512 if j < 2 else 256
                        nc.vector.tensor_sub(o_sb[:, 512 * j: 512 * j + w],
                                             o_ps[j][:, 0:w],
                                             b_sb[:, 512 * j: 512 * j + w])
                    # store each 128-row half as soon as its subs complete,
                    # alternating HWDGE (scalar) and SWDGE (gpsimd) queues
                    seng = nc.scalar if n == 0 else nc.gpsimd
                    r0 = 256 * tt + 128 * n
                    seng.dma_start(out[r0: r0 + 128, :], o_sb[:, 0:COLS])
    nc.finalize()
    return nc


def _host_fold(Cg, Sg, w4, w8, w16, w32, gate, noise_u):
    ws = {4: w4, 8: w8, 16: w16, 32: w32}
    C_f = np.zeros((F, 32, 32), np.float64)
    for f in range(F):
        g, a = f // 4, f % 4
        C_f[f] = Cg[32 * a:32 * a + 32, 128 * g + 32 * a:128 * g + 32 * a + 32]
    s_f = Sg.reshape(G * 4, 32)[:F].astype(np.float64)

    mu = np.zeros((4, E)); msq = np.zeros((4, E))
    for k, d in enumerate(IN_DIMS):
        w = ws[d].astype(np.float64)
        mu[k] = np.einsum('fi,fie->e', s_f[:, :d], w) / (B * F)
        msq[k] = np.einsum('fij,fie,fje->e', C_f[:, :d, :d], w, w) / (B * F)
    var = msq - mu ** 2
    rsig = 1.0 / np.sqrt(var + 1e-5)

    gmb = -np.log(-np.log(noise_u.astype(np.float64) + 1e-10) + 1e-10)
    z = (gate.astype(np.float64) + gmb)
    z -= z.max(axis=-1, keepdims=True)
    gs = np.exp(z) / np.exp(z).sum(axis=-1, keepdims=True)
    a_ = gs / 4.0

    Wc = np.zeros((F, 32, E), np.float64)
    bias = np.zeros((F, E), np.float64)
    for k, d in enumerate(IN_DIMS):
        w = ws[d].astype(np.float64)
        Wc[:, :d, :] += a_[:, k, None, None] * rsig[k][None, None, :] * w
        bias += a_[:, k, None] * (rsig[k] * mu[k])[None, :]

    Wbd = np.zeros((G, 128, 128), np.float32)
    bias_pc = np.zeros((128, PCOLS), np.float32)
    for f in range(F):
        g, a = f // 4, f % 4
        Wbd[g, 32 * a:32 * a + 32, 32 * a:32 * a + 32] = Wc[f]
        bias_pc[:, 128 * g + 32 * a: 128 * g + 32 * a + 32] = bias[f][None, :]
    return Wbd, bias_pc


def kernel(emb, w4, w8, w16, w32, gate, noise_u):
    import ml_dtypes
    emb = np.asarray(emb, np.float32).reshape(B, COLS)
    embp = np.zeros((B, PCOLS), ml_dtypes.bfloat16)
    embp[:, :COLS] = emb
    shards = embp.reshape(NC, BC, PCOLS)
    embp8 = np.zeros((B, PCOLS), ml_dtypes.float8_e4m3)
    embp8[:, :COLS] = emb
    shards8 = embp8.reshape(NC, BC, PCOLS)
    core_ids = list(range(NC))

    if "p1" not in _CACHE:
        _CACHE["p1"] = _build_phase1()
    ones_in = np.ones((128, 64), ml_dtypes.float8_e4m3)
    r1 = run_bass_kernel_spmd(
        _CACHE["p1"],
        [{"emb": shards8[c], "ones_in": ones_in} for c in range(NC)],
        core_ids,
    ).results
    Cg = np.zeros((128, PCOLS), np.float64)
    Sg = np.zeros((1, PCOLS), np.float64)
    for r in r1:
        Cg += r["c_out"]
        Sg += r["s_out"]

    Wbd, bias_pc = _host_fold(Cg, Sg, np.asarray(w4), np.asarray(w8),
                              np.asarray(w16), np.asarray(w32),
                              np.asarray(gate), np.asarray(noise_u))
    Wbd = np.ascontiguousarray(
        Wbd.transpose(1, 0, 2).reshape(128, G * 128)).astype(ml_dtypes.bfloat16)
    ident = np.eye(128, dtype=ml_dtypes.bfloat16)

    if "p2" not in _CACHE:
        _CACHE["p2"] = _build_phase2()
    r2 = run_bass_kernel_spmd(
        _CACHE["p2"],
        [{"emb": shards[c], "wbd": Wbd, "bias": bias_pc, "ident": ident}
         for c in range(NC)],
        core_ids,
    ).results
    out = np.concatenate([r["out"] for r in r2], axis=0)
    return out.reshape(B, F, E)
